# revision 26
# baseline (speedup 1.0000x reference)
"""Trainium2 Bass kernel for nn_Discriminator_455266534113 (relational GCN discriminator).

Data-parallel across 8 NeuronCores: batch 512 -> 64 per core. All weights replicated.

Layout strategy (host-side prep, device does zero transposes):
  - A [512,128,128,5] -> per core AT [64, n=128, r*128+m] in bf16 (adjacency pre-transposed
    so the contraction index n lands on SBUF partitions; contiguous DMA, half the HBM traffic)
  - X -> per core XG [16 groups, 33, 4*128] f32 (features transposed, 4 batch elems side by
    side, row 32 = ones so bias folds into the matmul contraction)
  - Per-relation weights concatenated along free dim with a bias row appended:
    WL1 [33, 5*64], WL2 [65, 5*128]; feature-branch WF1 [33,64], WF2 [65,128].

Precision strategy (validated numerically: final rel err ~1e-4 vs f32 reference):
  - Adjacency (uniform[0,1)) and post-relu h tensors in bf16 -> the 40 small (Nf=128)
    aggregation matmuls run at 1 cycle/row with fast weight load.
  - All wide matmuls (Nf>=256) use float32r operands (full fp32 bits, fast PE mode).
  - PSUM accumulation is always fp32; layer 1 is saturated (z1 > 46) so bf16 noise vanishes.

Per batch elem b (all "transposed" layout, features on partitions):
  h1 = relu(xT_aug.T @ WL1)              [n=128, 320]   one f32r matmul, bf16 eviction
  aggT1 slice = sum_r h1_r.T @ adjT_r  (+ relu(WF1.T @ xT) injected via identity matmul)
  x1T = tanh(aggT1)  -- one ACT op per group of 4, read straight from PSUM
  (same for layer 2 -> x2T), then gated aggregation batched 4-wide:
  iT = sigmoid(WI.T@x2T+bi); jT = tanh(WJ.T@iT+bj); g_raw = sum_n iT*jT (Pool mul + DVE reduce)
  head: tanh(g) -> W1 -> tanh -> W2 -> tanh -> out [1, 64] per core
"""

import os
import sys
from contextlib import ExitStack

import numpy as np

if "/opt/trn_rl_repo" not in sys.path:
    sys.path.insert(0, "/opt/trn_rl_repo")

B, N, R, F = 512, 128, 5, 32
H1, H2 = 64, 128
NCORES, BPC, G = 8, 64, 4
NG = BPC // G  # 16 groups per core

# Packed weight tensor column layout: name -> (rows, col0, width)
_W_SHAPES = [
    ("wl1", F + 1, R * H1), ("wf1", F + 1, H1), ("wl2", H1 + 1, R * H2),
    ("wf2", H1 + 1, H2), ("wi", H2, 128), ("wj", 128, 128), ("w1", 128, 128),
    ("bi", 128, 1), ("bj", 128, 1), ("b1", 128, 1), ("w2", 128, 1), ("b2", 1, 1),
]
WCOL = {}
_c = 0
for _nm, _rows, _w in _W_SHAPES:
    WCOL[_nm] = _c
    _c += _w
WPACK_W = _c
BCOL = {"wl1": 0, "wf1": R * H1}
BPACK_W = R * H1 + H1


def _build_nc(rep: int = 1, wide_dt: str = "all"):
    import concourse.bass as bass
    import concourse.mybir as mybir
    import concourse.tile as tile
    from concourse import bacc
    from concourse.masks import make_identity

    f32 = mybir.dt.float32
    bf16 = mybir.dt.bfloat16
    import os
    _mode = os.environ.get("WIDE_DT", wide_dt)  # all | layers | gated | none
    if _mode == "f32r":
        _mode = "all"
    if _mode == "f32":
        _mode = "none"
    f32r = mybir.dt.float32r
    l1bf = os.environ.get("L1BF16", "0") == "1"
    dt_l = mybir.dt.float32r if _mode in ("all", "layers") else mybir.dt.float32
    dt_g = mybir.dt.float32r if _mode in ("all", "gated") else mybir.dt.float32
    AF = mybir.ActivationFunctionType

    nc = bacc.Bacc("TRN2", target_bir_lowering=False, debug=False)

    AT = nc.dram_tensor("AT", [BPC, N, R * N], bf16, kind="ExternalInput").ap()
    XG = nc.dram_tensor("XG", [NG, F + 1, G * N], bf16 if l1bf else dt_l, kind="ExternalInput").ap()
    # All weights/biases packed into one tensor (one DMA): col layout see WPACK_COLS
    WPACK = nc.dram_tensor("WPACK", [128, WPACK_W], f32r, kind="ExternalInput").ap()
    WB16 = nc.dram_tensor("WB16", [F + 1, BPACK_W], bf16, kind="ExternalInput").ap()
    OUT = nc.dram_tensor("OUT", [1, BPC], f32, kind="ExternalOutput").ap()

    with tile.TileContext(nc) as tc, ExitStack() as ctx:
        const = ctx.enter_context(tc.tile_pool(name="const", bufs=1))
        a_pool = ctx.enter_context(tc.tile_pool(name="a_pool", bufs=3))
        xg_pool = ctx.enter_context(tc.tile_pool(name="xg_pool", bufs=2))
        h1_pool = ctx.enter_context(tc.tile_pool(name="h1_pool", bufs=4))
        h2_pool = ctx.enter_context(tc.tile_pool(name="h2_pool", bufs=4))
        f1_pool = ctx.enter_context(tc.tile_pool(name="f1_pool", bufs=3))
        f2_pool = ctx.enter_context(tc.tile_pool(name="f2_pool", bufs=3))
        x1_pool = ctx.enter_context(tc.tile_pool(name="x1_pool", bufs=3))
        x2_pool = ctx.enter_context(tc.tile_pool(name="x2_pool", bufs=3))
        i_pool = ctx.enter_context(tc.tile_pool(name="i_pool", bufs=2))
        j_pool = ctx.enter_context(tc.tile_pool(name="j_pool", bufs=2))
        p_pool = ctx.enter_context(tc.tile_pool(name="p_pool", bufs=2))

        # PSUM: 8 banks total -> 3 + 1 + 2 + 2
        ps_h = ctx.enter_context(tc.tile_pool(name="ps_h", bufs=3, space="PSUM"))
        ps_a1 = ctx.enter_context(tc.tile_pool(name="ps_a1", bufs=1, space="PSUM"))
        ps_a2 = ctx.enter_context(tc.tile_pool(name="ps_a2", bufs=2, space="PSUM"))
        ps_g = ctx.enter_context(tc.tile_pool(name="ps_g", bufs=2, space="PSUM"))

        wpack = const.tile([128, WPACK_W], f32r, tag="wpack")
        # L1-critical weights (wl1/wf1 cols) land first so group 0 starts sooner
        nc.sync.dma_start(wpack[:, 0:WCOL["wl2"]], WPACK[:, 0:WCOL["wl2"]])
        nc.sync.dma_start(wpack[:, WCOL["wl2"]:], WPACK[:, WCOL["wl2"]:])
        wb16_t = const.tile([F + 1, BPACK_W], bf16, tag="wb16")
        nc.sync.dma_start(wb16_t[:], WB16)
        wb16 = wb16_t[:]

        def wslice(rows, c0, w, dt):
            ap = wpack[0:rows, c0:c0 + w]
            return ap if dt is f32r else ap.bitcast(dt)

        if l1bf:
            wl1 = wb16[0:F + 1, BCOL["wl1"]:BCOL["wl1"] + R * H1]
            wf1 = wb16[0:F + 1, BCOL["wf1"]:BCOL["wf1"] + H1]
        else:
            wl1 = wslice(F + 1, WCOL["wl1"], R * H1, dt_l)
            wf1 = wslice(F + 1, WCOL["wf1"], H1, dt_l)
        wl2 = wslice(H1 + 1, WCOL["wl2"], R * H2, dt_l)
        wf2 = wslice(H1 + 1, WCOL["wf2"], H2, dt_l)
        wi = wslice(H2, WCOL["wi"], 128, dt_g)
        bi = wslice(128, WCOL["bi"], 1, f32)
        wj = wslice(128, WCOL["wj"], 128, dt_g)
        bj = wslice(128, WCOL["bj"], 1, f32)
        w1 = wslice(128, WCOL["w1"], 128, f32)
        b1 = wslice(128, WCOL["b1"], 1, f32)
        w2 = wslice(128, WCOL["w2"], 1, f32)
        b2 = wslice(1, WCOL["b2"], 1, f32)
        g_raw = const.tile([128, BPC], f32, tag="g_raw")
        i64 = const.tile([H1, H1], bf16, tag="i64")
        make_identity(nc, i64[:])
        i128 = const.tile([H2, H2], bf16, tag="i128")
        make_identity(nc, i128[:])

        def emit_tail(g, a2p):
            """Gated aggregation for group g — emitted one group late so its
            serial ACT/PE ping-pong overlaps the next group's dense work."""
            x2g = x2_pool.tile([H2, G * N], dt_g, tag="x2g")
            nc.scalar.activation(x2g[:], a2p[:], AF.Tanh)
            ip = ps_g.tile([128, G * N], f32, tag="psg")
            nc.tensor.matmul(ip[:], lhsT=wi, rhs=x2g[:], start=True, stop=True)
            is_ = i_pool.tile([128, G * N], dt_g, tag="is")
            nc.scalar.activation(is_[:], ip[:], AF.Sigmoid, bias=bi)
            jp = ps_g.tile([128, G * N], f32, tag="psg")
            nc.tensor.matmul(jp[:], lhsT=wj, rhs=is_[:], start=True, stop=True)
            js_t = j_pool.tile([128, G * N], f32, tag="js")
            nc.scalar.activation(js_t[:], jp[:], AF.Tanh, bias=bj)
            prod = p_pool.tile([128, G * N], f32, tag="prod")
            nc.gpsimd.tensor_mul(prod[:], is_[:].bitcast(f32), js_t[:])
            nc.vector.tensor_reduce(
                g_raw[:, G * g:G * (g + 1)],
                prod[:].rearrange("p (j n) -> p j n", n=N),
                axis=mybir.AxisListType.X,
                op=mybir.AluOpType.add,
            )

        def emit_L1(g):
            """DMAs + feat1 + per-j h1/agg1/inject + tanh -> returns (x1g, ats)."""
            xg = xg_pool.tile([F + 1, G * N], bf16 if l1bf else dt_l, tag="xg")
            nc.sync.dma_start(xg[:], XG[g])
            # all 4 adjacency tiles in one DMA (HWDGE descriptor cost is per dma_start)
            at_g = a_pool.tile([N, G * R * N], bf16, tag="at")
            nc.sync.dma_start(
                at_g[:].rearrange("n (j m) -> n j m", m=R * N),
                AT[G * g:G * (g + 1)].rearrange("j n m -> n j m"),
            )
            ats = [at_g[:, j * R * N:(j + 1) * R * N] for j in range(G)]

            f1p = ps_g.tile([H1, G * N], f32, tag="psg")
            nc.tensor.matmul(f1p[:], lhsT=wf1, rhs=xg[:], start=True, stop=True)
            f1s = f1_pool.tile([H1, G * N], bf16, tag="f1s")
            nc.scalar.activation(f1s[:], f1p[:], AF.Relu)

            x1g = x1_pool.tile([H1 + 1, G * N], dt_l, tag="x1g")
            nc.gpsimd.memset(x1g[H1:H1 + 1, :].bitcast(f32), 1.0)

            a1p = ps_a1.tile([H1, G * N], f32, tag="a1p")
            for j in range(G):
                js = slice(j * N, (j + 1) * N)
                h1p = ps_h.tile([N, R * H1], f32, tag="ph")
                nc.tensor.matmul(h1p[:], lhsT=xg[:, js], rhs=wl1, start=True, stop=True)
                h1s = h1_pool.tile([N, R * H1], bf16, tag="h1s")
                if j < 2:  # rebalance: DVE is the busiest engine, ACT has headroom
                    nc.scalar.activation(h1s[:], h1p[:], AF.Relu)
                else:
                    nc.vector.tensor_scalar_max(h1s[:], h1p[:], 0.0)
                for rr in range(R):
                    nc.tensor.matmul(
                        a1p[:, js],
                        lhsT=h1s[:, rr * H1:(rr + 1) * H1],
                        rhs=ats[j][:, rr * N:(rr + 1) * N],
                        start=(rr == 0),
                        stop=False,
                    )
                nc.tensor.matmul(a1p[:, js], lhsT=i64[:], rhs=f1s[:, js], start=False, stop=True)
            nc.scalar.activation(x1g[0:H1, :], a1p[:], AF.Tanh)
            return x1g, ats

        def emit_L2(x1g, ats):
            f2p = ps_g.tile([H2, G * N], f32, tag="psg")
            nc.tensor.matmul(f2p[:], lhsT=wf2, rhs=x1g[:], start=True, stop=True)
            f2s = f2_pool.tile([H2, G * N], bf16, tag="f2s")
            nc.scalar.activation(f2s[:], f2p[:], AF.Relu)

            a2p = ps_a2.tile([H2, G * N], f32, tag="a2p")
            for j in range(G):
                js = slice(j * N, (j + 1) * N)
                h2pa = ps_h.tile([N, 320], f32, tag="ph")
                nc.tensor.matmul(h2pa[:], lhsT=x1g[:, js], rhs=wl2[:, 0:320], start=True, stop=True)
                h2pb = ps_h.tile([N, 320], f32, tag="ph")
                nc.tensor.matmul(h2pb[:], lhsT=x1g[:, js], rhs=wl2[:, 320:640], start=True, stop=True)
                h2s = h2_pool.tile([N, R * H2], bf16, tag="h2s")
                nc.vector.tensor_scalar_max(h2s[:, 0:320], h2pa[:], 0.0)
                nc.vector.tensor_scalar_max(h2s[:, 320:640], h2pb[:], 0.0)
                for rr in range(R):
                    nc.tensor.matmul(
                        a2p[:, js],
                        lhsT=h2s[:, rr * H2:(rr + 1) * H2],
                        rhs=ats[j][:, rr * N:(rr + 1) * N],
                        start=(rr == 0),
                        stop=False,
                    )
                nc.tensor.matmul(a2p[:, js], lhsT=i128[:], rhs=f2s[:, js], start=False, stop=True)
            return a2p

        # Software pipeline: L1(g+1) is emitted before L2(g) so its PE/DVE work
        # fills the tanh-x1/f2-relu stalls; the gated tail runs one group late.
        total = NG * rep
        cur = emit_L1(0)
        pending = None
        for g in range(total):
            nxt = emit_L1((g + 1) % NG) if g + 1 < total else None
            a2p = emit_L2(*cur)
            if pending is not None:
                emit_tail(*pending)
            pending = (g % NG, a2p)
            cur = nxt
        emit_tail(*pending)

        # ---- head, once per core ----
        gt = const.tile([128, BPC], f32, tag="gt")
        nc.scalar.activation(gt[:], g_raw[:], AF.Tanh)
        hp = ps_g.tile([128, BPC], f32, tag="psg")
        nc.tensor.matmul(hp[:], lhsT=w1, rhs=gt[:], start=True, stop=True)
        hs = const.tile([128, BPC], f32, tag="hs")
        nc.scalar.activation(hs[:], hp[:], AF.Tanh, bias=b1)
        op = ps_g.tile([1, BPC], f32, tag="psg")
        nc.tensor.matmul(op[:], lhsT=w2, rhs=hs[:], start=True, stop=True)
        os_ = const.tile([1, BPC], f32, tag="os")
        nc.scalar.activation(os_[:], op[:], AF.Tanh, bias=b2)
        import os as _os
        if _os.environ.get("REP_MARKER", "0") == "1" and rep != 1:
            nc.scalar.mul(os_[:], os_[:], float(rep))
        nc.sync.dma_start(OUT, os_[:])

    nc.compile()
    return nc


_NC_CACHE = {}


def _get_nc(rep: int = 1):
    if rep not in _NC_CACHE:
        _NC_CACHE[rep] = _build_nc(rep)
    return _NC_CACHE[rep]


def host_prep(inputs):
    import ml_dtypes

    A = np.asarray(inputs["A"], dtype=np.float32)
    X = np.asarray(inputs["X"], dtype=np.float32)
    f32 = np.float32

    def arr(name):
        return np.ascontiguousarray(np.asarray(inputs[name], dtype=f32))

    Wl1, bl1 = arr("Wl1"), arr("bl1")
    Wf1, bf1 = arr("Wf1"), arr("bf1")
    Wl2, bl2 = arr("Wl2"), arr("bl2")
    Wf2, bf2 = arr("Wf2"), arr("bf2")

    wp = np.zeros((128, WPACK_W), np.float32)

    def put(nm, mat):
        rows, width = mat.shape
        wp[0:rows, WCOL[nm]:WCOL[nm] + width] = mat

    put("wl1", np.concatenate([Wl1.transpose(1, 0, 2).reshape(F, R * H1), bl1.reshape(1, R * H1)], 0))
    put("wf1", np.concatenate([Wf1, bf1[None]], 0))
    put("wl2", np.concatenate([Wl2.transpose(1, 0, 2).reshape(H1, R * H2), bl2.reshape(1, R * H2)], 0))
    put("wf2", np.concatenate([Wf2, bf2[None]], 0))
    put("wi", arr("Wi"))
    put("wj", arr("Wj"))
    put("w1", arr("W1"))
    put("bi", arr("bi").reshape(128, 1))
    put("bj", arr("bj").reshape(128, 1))
    put("b1", arr("b1").reshape(128, 1))
    put("w2", arr("W2"))
    put("b2", arr("b2").reshape(1, 1))
    import ml_dtypes
    wb = np.zeros((F + 1, BPACK_W), np.float32)
    wb[:, BCOL["wl1"]:BCOL["wl1"] + R * H1] = np.concatenate(
        [Wl1.transpose(1, 0, 2).reshape(F, R * H1), bl1.reshape(1, R * H1)], 0)
    wb[:, BCOL["wf1"]:BCOL["wf1"] + H1] = np.concatenate([Wf1, bf1[None]], 0)
    W = {"WPACK": wp, "WB16": wb.astype(ml_dtypes.bfloat16)}

    in_maps = []
    for c in range(NCORES):
        bs = slice(c * BPC, (c + 1) * BPC)
        AT = np.ascontiguousarray(
            A[bs].transpose(0, 2, 3, 1).reshape(BPC, N, R * N).astype(ml_dtypes.bfloat16)
        )
        Xt = (
            X[bs]
            .transpose(0, 2, 1)
            .reshape(NG, G, F, N)
            .transpose(0, 2, 1, 3)
            .reshape(NG, F, G * N)
        )
        XGa = np.concatenate([Xt, np.ones((NG, 1, G * N), f32)], 1)
        if os.environ.get("L1BF16", "0") == "1":
            XGa = XGa.astype(ml_dtypes.bfloat16)
        XGa = np.ascontiguousarray(XGa)
        in_maps.append({"AT": AT, "XG": XGa, **W})
    return in_maps


def kernel(**inputs) -> np.ndarray:
    from concourse.bass_utils import run_bass_kernel_spmd

    in_maps = host_prep(inputs)
    nc = _get_nc()
    res = run_bass_kernel_spmd(nc, in_maps, core_ids=list(range(NCORES)))
    out = np.concatenate([r["OUT"].reshape(BPC) for r in res.results])
    return out.reshape(B, 1).astype(np.float32)


# revision 28
# speedup vs baseline: 1.4516x; 1.4516x over previous
"""Trainium2 Bass kernel for nn_Discriminator_455266534113 (relational GCN discriminator).

Data-parallel across 8 NeuronCores: batch 512 -> 64 per core. All weights replicated.

Layout strategy (host-side prep, device does zero transposes):
  - A [512,128,128,5] -> per core AT [64, n=128, r*128+m] in bf16 (adjacency pre-transposed
    so the contraction index n lands on SBUF partitions; contiguous DMA, half the HBM traffic)
  - X -> per core XG [16 groups, 33, 4*128] f32 (features transposed, 4 batch elems side by
    side, row 32 = ones so bias folds into the matmul contraction)
  - Per-relation weights concatenated along free dim with a bias row appended:
    WL1 [33, 5*64], WL2 [65, 5*128]; feature-branch WF1 [33,64], WF2 [65,128].

Precision strategy (validated numerically: final rel err ~1e-4 vs f32 reference):
  - Adjacency (uniform[0,1)) and post-relu h tensors in bf16 -> the 40 small (Nf=128)
    aggregation matmuls run at 1 cycle/row with fast weight load.
  - All wide matmuls (Nf>=256) use float32r operands (full fp32 bits, fast PE mode).
  - PSUM accumulation is always fp32; layer 1 is saturated (z1 > 46) so bf16 noise vanishes.

Per batch elem b (all "transposed" layout, features on partitions):
  h1 = relu(xT_aug.T @ WL1)              [n=128, 320]   one f32r matmul, bf16 eviction
  aggT1 slice = sum_r h1_r.T @ adjT_r  (+ relu(WF1.T @ xT) injected via identity matmul)
  x1T = tanh(aggT1)  -- one ACT op per group of 4, read straight from PSUM
  (same for layer 2 -> x2T), then gated aggregation batched 4-wide:
  iT = sigmoid(WI.T@x2T+bi); jT = tanh(WJ.T@iT+bj); g_raw = sum_n iT*jT (Pool mul + DVE reduce)
  head: tanh(g) -> W1 -> tanh -> W2 -> tanh -> out [1, 64] per core
"""

import os
import sys
from contextlib import ExitStack

import numpy as np

if "/opt/trn_rl_repo" not in sys.path:
    sys.path.insert(0, "/opt/trn_rl_repo")

B, N, R, F = 512, 128, 5, 32
H1, H2 = 64, 128
NCORES, BPC, G = 8, 64, 4
NG = BPC // G  # 16 groups per core

# Packed weight tensor column layout: name -> (rows, col0, width)
_W_SHAPES = [
    ("wl1", F + 1, R * H1), ("wf1", F + 1, H1), ("wl2", H1 + 1, R * H2),
    ("wf2", H1 + 1, H2), ("wi", H2, 128), ("wj", 128, 128), ("w1", 128, 128),
    ("bi", 128, 1), ("bj", 128, 1), ("b1", 128, 1), ("w2", 128, 1), ("b2", 1, 1),
]
WCOL = {}
_c = 0
for _nm, _rows, _w in _W_SHAPES:
    WCOL[_nm] = _c
    _c += _w
WPACK_W = _c
BCOL = {"wl1": 0, "wf1": R * H1}
BPACK_W = R * H1 + H1


def _build_nc(rep: int = 1, wide_dt: str = "all"):
    import concourse.bass as bass
    import concourse.mybir as mybir
    import concourse.tile as tile
    from concourse import bacc
    from concourse.masks import make_identity

    f32 = mybir.dt.float32
    bf16 = mybir.dt.bfloat16
    import os
    _mode = os.environ.get("WIDE_DT", wide_dt)  # all | layers | gated | none
    if _mode == "f32r":
        _mode = "all"
    if _mode == "f32":
        _mode = "none"
    f32r = mybir.dt.float32r
    l1bf = os.environ.get("L1BF16", "0") == "1"
    dt_l = mybir.dt.float32r if _mode in ("all", "layers") else mybir.dt.float32
    dt_g = mybir.dt.float32r if _mode in ("all", "gated") else mybir.dt.float32
    AF = mybir.ActivationFunctionType

    nc = bacc.Bacc("TRN2", target_bir_lowering=False, debug=False)

    AT = nc.dram_tensor("AT", [BPC, N, R * N], bf16, kind="ExternalInput").ap()
    XG = nc.dram_tensor("XG", [NG, F + 1, G * N], bf16 if l1bf else dt_l, kind="ExternalInput").ap()
    # All weights/biases packed into one tensor (one DMA): col layout see WPACK_COLS
    WPACK = nc.dram_tensor("WPACK", [128, WPACK_W], f32r, kind="ExternalInput").ap()
    WB16 = nc.dram_tensor("WB16", [F + 1, BPACK_W], bf16, kind="ExternalInput").ap()
    OUT = nc.dram_tensor("OUT", [1, BPC], f32, kind="ExternalOutput").ap()

    with tile.TileContext(nc) as tc, ExitStack() as ctx:
        const = ctx.enter_context(tc.tile_pool(name="const", bufs=1))
        a_pool = ctx.enter_context(tc.tile_pool(name="a_pool", bufs=3))
        xg_pool = ctx.enter_context(tc.tile_pool(name="xg_pool", bufs=2))
        h1_pool = ctx.enter_context(tc.tile_pool(name="h1_pool", bufs=4))
        h2_pool = ctx.enter_context(tc.tile_pool(name="h2_pool", bufs=4))
        f1_pool = ctx.enter_context(tc.tile_pool(name="f1_pool", bufs=3))
        f2_pool = ctx.enter_context(tc.tile_pool(name="f2_pool", bufs=3))
        x1_pool = ctx.enter_context(tc.tile_pool(name="x1_pool", bufs=3))
        x2_pool = ctx.enter_context(tc.tile_pool(name="x2_pool", bufs=3))
        i_pool = ctx.enter_context(tc.tile_pool(name="i_pool", bufs=2))
        j_pool = ctx.enter_context(tc.tile_pool(name="j_pool", bufs=2))
        p_pool = ctx.enter_context(tc.tile_pool(name="p_pool", bufs=2))

        # PSUM: 8 banks total -> 3 + 1 + 2 + 2
        ps_h = ctx.enter_context(tc.tile_pool(name="ps_h", bufs=3, space="PSUM"))
        ps_a1 = ctx.enter_context(tc.tile_pool(name="ps_a1", bufs=1, space="PSUM"))
        ps_a2 = ctx.enter_context(tc.tile_pool(name="ps_a2", bufs=2, space="PSUM"))
        ps_g = ctx.enter_context(tc.tile_pool(name="ps_g", bufs=2, space="PSUM"))

        wpack = const.tile([128, WPACK_W], f32r, tag="wpack")
        # L1-critical weights (wl1/wf1 cols) land first so group 0 starts sooner
        nc.sync.dma_start(wpack[:, 0:WCOL["wl2"]], WPACK[:, 0:WCOL["wl2"]])
        nc.sync.dma_start(wpack[:, WCOL["wl2"]:], WPACK[:, WCOL["wl2"]:])
        wb16_t = const.tile([F + 1, BPACK_W], bf16, tag="wb16")
        nc.sync.dma_start(wb16_t[:], WB16)
        wb16 = wb16_t[:]

        def wslice(rows, c0, w, dt):
            ap = wpack[0:rows, c0:c0 + w]
            return ap if dt is f32r else ap.bitcast(dt)

        if l1bf:
            wl1 = wb16[0:F + 1, BCOL["wl1"]:BCOL["wl1"] + R * H1]
            wf1 = wb16[0:F + 1, BCOL["wf1"]:BCOL["wf1"] + H1]
        else:
            wl1 = wslice(F + 1, WCOL["wl1"], R * H1, dt_l)
            wf1 = wslice(F + 1, WCOL["wf1"], H1, dt_l)
        wl2 = wslice(H1 + 1, WCOL["wl2"], R * H2, dt_l)
        wf2 = wslice(H1 + 1, WCOL["wf2"], H2, dt_l)
        wi = wslice(H2, WCOL["wi"], 128, dt_g)
        bi = wslice(128, WCOL["bi"], 1, f32)
        wj = wslice(128, WCOL["wj"], 128, dt_g)
        bj = wslice(128, WCOL["bj"], 1, f32)
        w1 = wslice(128, WCOL["w1"], 128, f32)
        b1 = wslice(128, WCOL["b1"], 1, f32)
        w2 = wslice(128, WCOL["w2"], 1, f32)
        b2 = wslice(1, WCOL["b2"], 1, f32)
        g_raw = const.tile([128, BPC], f32, tag="g_raw")
        i64 = const.tile([H1, H1], bf16, tag="i64")
        make_identity(nc, i64[:])
        i128 = const.tile([H2, H2], bf16, tag="i128")
        make_identity(nc, i128[:])

        def emit_tail(g, a2p):
            """Gated aggregation for group g — emitted one group late so its
            serial ACT/PE ping-pong overlaps the next group's dense work."""
            x2g = x2_pool.tile([H2, G * N], dt_g, tag="x2g")
            nc.scalar.activation(x2g[:], a2p[:], AF.Tanh)
            ip = ps_g.tile([128, G * N], f32, tag="psg")
            nc.tensor.matmul(ip[:], lhsT=wi, rhs=x2g[:], start=True, stop=True)
            is_ = i_pool.tile([128, G * N], dt_g, tag="is")
            nc.scalar.activation(is_[:], ip[:], AF.Sigmoid, bias=bi)
            jp = ps_g.tile([128, G * N], f32, tag="psg")
            nc.tensor.matmul(jp[:], lhsT=wj, rhs=is_[:], start=True, stop=True)
            js_t = j_pool.tile([128, G * N], f32, tag="js")
            nc.scalar.activation(js_t[:], jp[:], AF.Tanh, bias=bj)
            prod = p_pool.tile([128, G * N], f32, tag="prod")
            nc.gpsimd.tensor_mul(prod[:], is_[:].bitcast(f32), js_t[:])
            nc.vector.tensor_reduce(
                g_raw[:, G * g:G * (g + 1)],
                prod[:].rearrange("p (j n) -> p j n", n=N),
                axis=mybir.AxisListType.X,
                op=mybir.AluOpType.add,
            )

        def emit_L1(g):
            """DMAs + feat1 + per-j h1/agg1/inject + tanh -> returns (x1g, ats)."""
            xg = xg_pool.tile([F + 1, G * N], bf16 if l1bf else dt_l, tag="xg")
            nc.sync.dma_start(xg[:], XG[g])
            # all 4 adjacency tiles in one DMA (HWDGE descriptor cost is per dma_start)
            at_g = a_pool.tile([N, G * R * N], bf16, tag="at")
            nc.sync.dma_start(
                at_g[:].rearrange("n (j m) -> n j m", m=R * N),
                AT[G * g:G * (g + 1)].rearrange("j n m -> n j m"),
            )
            ats = [at_g[:, j * R * N:(j + 1) * R * N] for j in range(G)]

            f1p = ps_g.tile([H1, G * N], f32, tag="psg")
            nc.tensor.matmul(f1p[:], lhsT=wf1, rhs=xg[:], start=True, stop=True)
            f1s = f1_pool.tile([H1, G * N], bf16, tag="f1s")
            nc.scalar.activation(f1s[:], f1p[:], AF.Relu)

            x1g = x1_pool.tile([H1 + 1, G * N], dt_l, tag="x1g")
            nc.gpsimd.memset(x1g[H1:H1 + 1, :].bitcast(f32), 1.0)

            a1p = ps_a1.tile([H1, G * N], f32, tag="a1p")
            for j in range(G):
                js = slice(j * N, (j + 1) * N)
                h1p = ps_h.tile([N, R * H1], f32, tag="ph")
                nc.tensor.matmul(h1p[:], lhsT=xg[:, js], rhs=wl1, start=True, stop=True)
                h1s = h1_pool.tile([N, R * H1], bf16, tag="h1s")
                if j < 2:  # rebalance: DVE is the busiest engine, ACT has headroom
                    nc.scalar.activation(h1s[:], h1p[:], AF.Relu)
                else:
                    nc.vector.tensor_scalar_max(h1s[:], h1p[:], 0.0)
                for rr in range(R):
                    nc.tensor.matmul(
                        a1p[:, js],
                        lhsT=h1s[:, rr * H1:(rr + 1) * H1],
                        rhs=ats[j][:, rr * N:(rr + 1) * N],
                        start=(rr == 0),
                        stop=False,
                    )
                nc.tensor.matmul(a1p[:, js], lhsT=i64[:], rhs=f1s[:, js], start=False, stop=True,
                                 skip_group_check=True)
            nc.scalar.activation(x1g[0:H1, :], a1p[:], AF.Tanh)
            return x1g, ats

        def emit_L2(x1g, ats):
            f2p = ps_g.tile([H2, G * N], f32, tag="psg")
            nc.tensor.matmul(f2p[:], lhsT=wf2, rhs=x1g[:], start=True, stop=True)
            f2s = f2_pool.tile([H2, G * N], bf16, tag="f2s")
            nc.scalar.activation(f2s[:], f2p[:], AF.Relu)

            a2p = ps_a2.tile([H2, G * N], f32, tag="a2p")
            for j in range(G):
                js = slice(j * N, (j + 1) * N)
                h2pa = ps_h.tile([N, 320], f32, tag="ph")
                nc.tensor.matmul(h2pa[:], lhsT=x1g[:, js], rhs=wl2[:, 0:320], start=True, stop=True)
                h2pb = ps_h.tile([N, 320], f32, tag="ph")
                nc.tensor.matmul(h2pb[:], lhsT=x1g[:, js], rhs=wl2[:, 320:640], start=True, stop=True)
                h2s = h2_pool.tile([N, R * H2], bf16, tag="h2s")
                nc.vector.tensor_scalar_max(h2s[:, 0:320], h2pa[:], 0.0)
                nc.vector.tensor_scalar_max(h2s[:, 320:640], h2pb[:], 0.0)
                for rr in range(R):
                    nc.tensor.matmul(
                        a2p[:, js],
                        lhsT=h2s[:, rr * H2:(rr + 1) * H2],
                        rhs=ats[j][:, rr * N:(rr + 1) * N],
                        start=(rr == 0),
                        stop=False,
                    )
                nc.tensor.matmul(a2p[:, js], lhsT=i128[:], rhs=f2s[:, js], start=False, stop=True,
                                 skip_group_check=True)
            return a2p

        # Software pipeline: L1(g+1) is emitted before L2(g) so its PE/DVE work
        # fills the tanh-x1/f2-relu stalls; the gated tail runs one group late.
        total = NG * rep
        cur = emit_L1(0)
        pending = None
        for g in range(total):
            nxt = emit_L1((g + 1) % NG) if g + 1 < total else None
            a2p = emit_L2(*cur)
            if pending is not None:
                emit_tail(*pending)
            pending = (g % NG, a2p)
            cur = nxt
        emit_tail(*pending)

        # ---- head, once per core ----
        gt = const.tile([128, BPC], f32, tag="gt")
        nc.scalar.activation(gt[:], g_raw[:], AF.Tanh)
        hp = ps_g.tile([128, BPC], f32, tag="psg")
        nc.tensor.matmul(hp[:], lhsT=w1, rhs=gt[:], start=True, stop=True)
        hs = const.tile([128, BPC], f32, tag="hs")
        nc.scalar.activation(hs[:], hp[:], AF.Tanh, bias=b1)
        op = ps_g.tile([1, BPC], f32, tag="psg")
        nc.tensor.matmul(op[:], lhsT=w2, rhs=hs[:], start=True, stop=True)
        os_ = const.tile([1, BPC], f32, tag="os")
        nc.scalar.activation(os_[:], op[:], AF.Tanh, bias=b2)
        import os as _os
        if _os.environ.get("REP_MARKER", "0") == "1" and rep != 1:
            nc.scalar.mul(os_[:], os_[:], float(rep))
        nc.sync.dma_start(OUT, os_[:])

    nc.compile()
    return nc


_NC_CACHE = {}


def _get_nc(rep: int = 1):
    if rep not in _NC_CACHE:
        _NC_CACHE[rep] = _build_nc(rep)
    return _NC_CACHE[rep]


def host_prep(inputs):
    import ml_dtypes

    A = np.asarray(inputs["A"], dtype=np.float32)
    X = np.asarray(inputs["X"], dtype=np.float32)
    f32 = np.float32

    def arr(name):
        return np.ascontiguousarray(np.asarray(inputs[name], dtype=f32))

    Wl1, bl1 = arr("Wl1"), arr("bl1")
    Wf1, bf1 = arr("Wf1"), arr("bf1")
    Wl2, bl2 = arr("Wl2"), arr("bl2")
    Wf2, bf2 = arr("Wf2"), arr("bf2")

    wp = np.zeros((128, WPACK_W), np.float32)

    def put(nm, mat):
        rows, width = mat.shape
        wp[0:rows, WCOL[nm]:WCOL[nm] + width] = mat

    put("wl1", np.concatenate([Wl1.transpose(1, 0, 2).reshape(F, R * H1), bl1.reshape(1, R * H1)], 0))
    put("wf1", np.concatenate([Wf1, bf1[None]], 0))
    put("wl2", np.concatenate([Wl2.transpose(1, 0, 2).reshape(H1, R * H2), bl2.reshape(1, R * H2)], 0))
    put("wf2", np.concatenate([Wf2, bf2[None]], 0))
    put("wi", arr("Wi"))
    put("wj", arr("Wj"))
    put("w1", arr("W1"))
    put("bi", arr("bi").reshape(128, 1))
    put("bj", arr("bj").reshape(128, 1))
    put("b1", arr("b1").reshape(128, 1))
    put("w2", arr("W2"))
    put("b2", arr("b2").reshape(1, 1))
    import ml_dtypes
    wb = np.zeros((F + 1, BPACK_W), np.float32)
    wb[:, BCOL["wl1"]:BCOL["wl1"] + R * H1] = np.concatenate(
        [Wl1.transpose(1, 0, 2).reshape(F, R * H1), bl1.reshape(1, R * H1)], 0)
    wb[:, BCOL["wf1"]:BCOL["wf1"] + H1] = np.concatenate([Wf1, bf1[None]], 0)
    W = {"WPACK": wp, "WB16": wb.astype(ml_dtypes.bfloat16)}

    in_maps = []
    for c in range(NCORES):
        bs = slice(c * BPC, (c + 1) * BPC)
        AT = np.ascontiguousarray(
            A[bs].transpose(0, 2, 3, 1).reshape(BPC, N, R * N).astype(ml_dtypes.bfloat16)
        )
        Xt = (
            X[bs]
            .transpose(0, 2, 1)
            .reshape(NG, G, F, N)
            .transpose(0, 2, 1, 3)
            .reshape(NG, F, G * N)
        )
        XGa = np.concatenate([Xt, np.ones((NG, 1, G * N), f32)], 1)
        if os.environ.get("L1BF16", "0") == "1":
            XGa = XGa.astype(ml_dtypes.bfloat16)
        XGa = np.ascontiguousarray(XGa)
        in_maps.append({"AT": AT, "XG": XGa, **W})
    return in_maps


def kernel(**inputs) -> np.ndarray:
    from concourse.bass_utils import run_bass_kernel_spmd

    in_maps = host_prep(inputs)
    nc = _get_nc()
    res = run_bass_kernel_spmd(nc, in_maps, core_ids=list(range(NCORES)))
    out = np.concatenate([r["OUT"].reshape(BPC) for r in res.results])
    return out.reshape(B, 1).astype(np.float32)
